# revision 32
# baseline (speedup 1.0000x reference)
"""GQA causal attention block (RoPE, 32 Q heads / 8 KV heads, S=2048, D=4096)
distributed tensor-parallel over heads across 8 TRN2 NeuronCores.

Per core c:
  - 4 query heads (wq cols 512c:512c+512), 1 KV head (wk/wv cols 128c:128c+128)
  - projections computed transposed (qT/kT/vT [hd, seq]) from host-transposed
    xT, weights as stationary operands, bf16 matmuls with f32 PSUM accum
  - RoPE applied with host tables (bf16); the half-rotation uses a PE
    permutation matmul (cross-partition moves are impossible on DVE)
  - attention group g (i-chunk g of 512) runs immediately after projection
    chunk g — causality means it only needs K/V through chunk g — so each
    chunked AllGather launches hundreds of us earlier and inter-core launch
    skew is absorbed by the remaining compute instead of stalling the PE
  - attention computed as S^T [j, i] tiles -> exp (ACT, scale folded in) ->
    P^T tiles feed the PV matmul directly; causal handled by skipping future
    j-tiles, streaming only the live column subrange on diagonal tiles, and
    one DVE 0/1 triangular mask multiply on the first 128 live columns; the
    l/oT accumulation trails st/exp by 3 tiles so the PE never blocks on the
    ACT exp latency
  - row sums via a ones-vector matmul; 1/l = exp(-ln l) computed entirely on
    ACT (keeps the in-order ACT queue free of cross-engine waits), broadcast
    across partitions with a K=1 PE matmul (no DRAM round-trip); the
    broadcast matmul is software-pipelined one head late
  - output projection computed transposed (outT = wo_c^T @ Y^T); outT is
    written bf16 and upcast on host
  - bulk DMA rides both the SP and (post-barrier) GpSimd queues, ordered so
    chunk-0 operands land first
Host gathers by concatenating the 8 (transposed) column shards.
"""

import numpy as np
import ml_dtypes

import concourse.bass as bass
import concourse.mybir as mybir
import concourse.tile as tile
from concourse.bass_utils import run_bass_kernel_spmd

bf16 = mybir.dt.bfloat16
f32 = mybir.dt.float32

NCORES = 8
S = 2048
DIM = 4096
HD = 128
NH = 32
QH = NH // NCORES          # 4 query heads per core
QW = QH * HD               # 512 wq cols per core
ROPE_BASE = 10000.0
SCALE = float(HD) ** -0.5
NSEQ = S // 512            # 4 seq chunks
KT = DIM // 128            # 32 contraction tiles

_CACHE = {}


def _legalize_waits(nc, allowed_default=1):
    """This walrus build rejects instructions carrying more inline sync waits
    than the opcode template allows (0 for Drain, 1 elsewhere). Spill excess
    waits onto standalone EventSemaphore instructions inserted immediately
    before, on the same engine (engine order preserves semantics)."""
    for f in nc.m.functions:
        for bb in f.blocks:
            out = []
            for ins in bb.instructions:
                tname = type(ins).__name__
                si = getattr(ins, "sync_info", None)
                waits = list(si.on_wait) if (si is not None and si.on_wait) else []
                if tname == "InstEventSemaphore":
                    allowed = len(waits)
                elif tname == "InstDrain":
                    allowed = 0
                else:
                    allowed = allowed_default
                if len(waits) > allowed:
                    spill, keep = waits[allowed:], waits[:allowed]
                    for i, w in enumerate(spill):
                        ev = mybir.InstEventSemaphore(
                            name=f"{ins.name}_wfix{i}",
                            engine=ins.engine, ins=[], outs=[],
                        )
                        ev.sync_info = mybir.SyncInfo(on_wait=[w], on_update=[])
                        out.append(ev)
                    si.on_wait = keep
                out.append(ins)
            bb.instructions[:] = out


def _build_nc():
    nc = bass.Bass(num_devices=NCORES)

    xT = nc.declare_dram_parameter("xT", [DIM, S], bf16, isOutput=False)
    wq = nc.declare_dram_parameter("wq", [DIM, QW], bf16, isOutput=False)
    wk = nc.declare_dram_parameter("wk", [DIM, HD], bf16, isOutput=False)
    wv = nc.declare_dram_parameter("wv", [DIM, HD], bf16, isOutput=False)
    wo = nc.declare_dram_parameter("wo", [DIM, QW], bf16, isOutput=False)
    cosT = nc.declare_dram_parameter("cosT", [HD, S], bf16, isOutput=False)
    sinT = nc.declare_dram_parameter("sinT", [HD, S], bf16, isOutput=False)
    tri = nc.declare_dram_parameter("tri", [128, 128], bf16, isOutput=False)
    perm = nc.declare_dram_parameter("perm", [128, 128], bf16, isOutput=False)
    ident = nc.declare_dram_parameter("ident", [128, 128], bf16, isOutput=False)
    outT = nc.declare_dram_parameter("outT", [QW, S], bf16, isOutput=True)

    ag_ins = [nc.dram_tensor(f"ag_in{g}", [QW, 512], bf16) for g in range(4)]
    ag_outs = [
        nc.dram_tensor(f"ag_out{g}", [NCORES, QW, 512], bf16, addr_space="Shared")
        for g in range(4)
    ]

    with tile.TileContext(nc) as tc:
        with (
            tc.tile_pool(name="const", bufs=1) as constp,
            tc.tile_pool(name="acts", bufs=1) as acts,
            tc.tile_pool(name="xin", bufs=6) as xin,
            tc.tile_pool(name="rope", bufs=2) as rope,
            tc.tile_pool(name="pt", bufs=6) as ptp,
            tc.tile_pool(name="epi", bufs=3) as epi,
            tc.tile_pool(name="cproj", bufs=12) as cproj,
            tc.tile_pool(name="psum", bufs=1, space="PSUM") as psum,
        ):
            def pbank(b, shape=(128, 512), dtype=f32, name="ps"):
                return psum.tile(list(shape), dtype, tag=f"b{b}",
                                 name=f"{name}_b{b}", bufs=1)

            def pbank01(name="ps"):
                # banks 0+1 fused as one [128,1024] tile: a single ACT
                # instruction can then exp two adjacent st tiles at once
                return psum.tile([128, 1024], f32, tag="b01",
                                 name=f"{name}_b01", bufs=1)

            # --- constants / weights (SBUF homes) ---
            wq_sb = constp.tile([128, KT, QW], bf16)
            wk_sb = constp.tile([128, KT, HD], bf16)
            wv_sb = constp.tile([128, KT, HD], bf16)
            wo_sb = constp.tile([128, KT, QW], bf16)
            cos_sb = constp.tile([HD, S], bf16)
            sin_sb = constp.tile([HD, S], bf16)
            tri_sb = constp.tile([128, 128], bf16)
            perm_sb = constp.tile([128, 128], bf16)
            ident_sb = constp.tile([128, 128], bf16)
            ones_sb = constp.tile([128, 1], bf16)
            onesr_sb = constp.tile([1, 128], bf16)
            nc.vector.memset(ones_sb[:], 1.0)
            nc.vector.memset(onesr_sb[:], 1.0)

            wqr = wq.rearrange("(a p) m -> p a m", p=128)
            wkr = wk.rearrange("(a p) m -> p a m", p=128)
            wvr = wv.rearrange("(a p) m -> p a m", p=128)
            wor = wo.rearrange("(a p) m -> p a m", p=128)
            xr = xT.rearrange("(a p) m -> p a m", p=128)

            # activations that live through attention
            qTr = acts.tile([128, QH, S], bf16)      # 4 head tiles [hd, seq]
            kTr = acts.tile([128, S], bf16)
            v_sb = acts.tile([128, S], bf16)         # 16 [seq,hd] tiles at jt*128

            # deferred per-head epilogue (bc matmul + normalize + store), run
            # one head late so the PE never waits on the 1/l ACT chain
            pending = []

            def flush_epilogue():
                if not pending:
                    return
                pg, ph, p_oT, p_linv = pending.pop()
                bc_ps = pbank(7, name="bc")
                nc.tensor.matmul(bc_ps[:], onesr_sb[:], p_linv[:],
                                 start=True, stop=True)
                bc_sb = epi.tile([128, 512], bf16, tag="bcsb")
                # copy on DVE: the ACT queue is the attention pacer (exps)
                nc.vector.tensor_copy(out=bc_sb[:], in_=bc_ps[:])
                oT_sb = epi.tile([128, 512], bf16, tag="otsb", bufs=6)
                nc.vector.tensor_mul(oT_sb[:], p_oT[:], bc_sb[:])
                nc.sync.dma_start(ag_ins[pg][bass.ts(ph, 128)], oT_sb[:])
                if ph == QH - 1:
                    nc.gpsimd.collective_compute(
                        "AllGather", mybir.AluOpType.bypass,
                        replica_groups=[list(range(NCORES))],
                        ins=[ag_ins[pg][:]], outs=[ag_outs[pg][:]],
                    )

            # ---- fused phase A+B: per chunk n, projections + rope, then
            # ---- attention group g=n (needs only K/V through chunk n)
            for n in range(NSEQ):
                sl = bass.ts(n, 512)
                q01 = pbank01(name="q")
                q_slots = [(q01, 0), (q01, 512), (pbank(2, name="q"), 0),
                           (pbank(3, name="q"), 0)]
                k_ps = pbank(4, name="k")
                vT_ps = pbank(5, name="vT")
                if n == 0:
                    # fine-grained wq/x interleave on SP so tile k lands in
                    # time for the k-th accumulation step; rope tables and
                    # small constants slotted in behind the first pairs
                    for k in range(KT):
                        if k in (0, 2, 4, 6):
                            # wk/wv in 256KB pieces, on SP: the GpSimd queue
                            # sits behind the startup barrier
                            p = k // 2
                            nc.sync.dma_start(wk_sb[:, 8 * p:8 * p + 8],
                                              wkr[:, 8 * p:8 * p + 8])
                            nc.sync.dma_start(wv_sb[:, 8 * p:8 * p + 8],
                                              wvr[:, 8 * p:8 * p + 8])
                        x_sb = xin.tile([128, 512], bf16, tag="x", bufs=10)
                        xq = (nc.sync, nc.scalar)[k % 2] if k < 12 else nc.sync
                        xq.dma_start(x_sb[:], xr[:, k, sl])
                        nc.sync.dma_start(wq_sb[:, k], wqr[:, k])
                        if k == 1:
                            nc.sync.dma_start(cos_sb[:, 0:1024], cosT[:, 0:1024])
                            nc.sync.dma_start(cos_sb[:, 1024:2048],
                                              cosT[:, 1024:2048])
                            nc.sync.dma_start(sin_sb[:, 0:1024], sinT[:, 0:1024])
                            nc.sync.dma_start(sin_sb[:, 1024:2048],
                                              sinT[:, 1024:2048])
                            nc.sync.dma_start(perm_sb[:], perm[:])
                        if k == 8:
                            nc.sync.dma_start(ident_sb[:], ident[:])
                            nc.sync.dma_start(tri_sb[:], tri[:])
                        st, sp = (k == 0), (k == KT - 1)
                        for m in range(QH):
                            qt, qb = q_slots[m]
                            nc.tensor.matmul(qt[:, qb:qb + 512],
                                             wq_sb[:, k, bass.ts(m, 128)],
                                             x_sb[:], start=st, stop=sp)
                        nc.tensor.matmul(k_ps[:], wk_sb[:, k], x_sb[:],
                                         start=st, stop=sp)
                        nc.tensor.matmul(vT_ps[:], wv_sb[:, k], x_sb[:],
                                         start=st, stop=sp)
                else:
                    # batched x: 16 pieces of 2 k-tiles (256KB each), issued
                    # alternately from SP and GpSimd (idle after the startup
                    # barrier) so neither queue paces the PE
                    for kb in range(16):
                        # SP/ACT alternation only: the GpSimd queue must stay
                        # empty of DMAs — an AllGather trigger parked there
                        # (waiting for the serial CC stream under launch skew)
                        # would head-of-line-block them and starve the PE
                        x2 = xin.tile([128, 2, 512], bf16, tag="x2", bufs=8)
                        eng = nc.sync if kb % 2 == 0 else nc.scalar
                        eng.dma_start(x2[:], xr[:, 2 * kb:2 * kb + 2, sl])
                        for j in range(2):
                            k = 2 * kb + j
                            st, sp = (k == 0), (k == KT - 1)
                            for m in range(QH):
                                qt, qb = q_slots[m]
                                nc.tensor.matmul(qt[:, qb:qb + 512],
                                                 wq_sb[:, k, bass.ts(m, 128)],
                                                 x2[:, j], start=st, stop=sp)
                            if k == 0:
                                # previous group's last epilogue: the k=0 q
                                # matmuls above give the PE runway for its 1/l
                                # chain; it must flush before the k/v matmuls
                                # below reuse PSUM banks 4/5
                                flush_epilogue()
                            nc.tensor.matmul(k_ps[:], wk_sb[:, k], x2[:, j],
                                             start=st, stop=sp)
                            nc.tensor.matmul(vT_ps[:], wv_sb[:, k], x2[:, j],
                                             start=st, stop=sp)
                    if n == 1:
                        # wo (needed by phase C) behind chunk-1 x traffic
                        for p in range(8):
                            nc.sync.dma_start(wo_sb[:, 4 * p:4 * p + 4],
                                              wor[:, 4 * p:4 * p + 4])

                # rope: q0 first (attention head 0 needs it first), then k
                # (needed by head 0's last j-tiles), then q1..q3; per tensor,
                # first free the accumulation bank (copy + cos-mul), then the
                # sw-product and adds
                order = [0, QH] + list(range(1, QH))   # q0, k, q1, q2, q3
                t_bfs, t1s = {}, {}
                for idx in order:
                    if idx < QH:
                        qt, qb = q_slots[idx]
                        src = qt[:, qb:qb + 512]
                    else:
                        src = k_ps[:]
                    t_bf = rope.tile([128, 512], bf16, tag=f"tbf{idx}",
                                     name=f"tbf{idx}", bufs=1)
                    nc.scalar.copy(t_bf[:], src)
                    t1 = rope.tile([128, 512], f32, tag=f"t1_{idx}",
                                   name=f"t1_{idx}", bufs=1)
                    nc.vector.tensor_mul(t1[:], src, cos_sb[:, sl])
                    t_bfs[idx] = t_bf
                    t1s[idx] = t1
                for i, idx in enumerate(order):
                    dst = qTr[:, idx, sl] if idx < QH else kTr[:, sl]
                    sw_ps = pbank(6 + (i % 2), name="sw")
                    nc.tensor.matmul(sw_ps[:], perm_sb[:], t_bfs[idx][:],
                                     start=True, stop=True)
                    t2 = rope.tile([128, 512], f32, tag=f"t2_{i % 2}",
                                   name=f"t2_{i % 2}")
                    nc.vector.tensor_mul(t2[:], sw_ps[:], sin_sb[:, sl])
                    nc.vector.tensor_add(dst, t1s[idx][:], t2[:])

                # v: copy vT chunk, transpose 128-blocks into [seq, hd] tiles
                v_bf = rope.tile([128, 512], bf16, tag="vbf")
                nc.scalar.copy(v_bf[:], vT_ps[:])
                for t in range(4):
                    vt_ps = pbank(6 + (t % 2), shape=(128, 128), dtype=bf16,
                                  name="vt")
                    nc.tensor.transpose(vt_ps[:], v_bf[:, bass.ts(t, 128)],
                                        ident_sb[:])
                    nc.any.tensor_copy(out=v_sb[:, bass.ts(4 * n + t, 128)],
                                       in_=vt_ps[:])

                # ---- attention group g = n ----
                # banks: st rotation 0/1/2, oT 3/4, l 5/6, bc 7
                g = n
                njt = 4 * g + 4
                for h in range(QH):
                    oT_ps = pbank(3 if h % 2 == 0 else 4, name="oT")
                    l_ps = pbank(5 if h % 2 == 0 else 6, shape=(1, 512), name="l")
                    LAG = 4   # l/oT consume pt behind st/exp so the PE never
                    #           blocks on the ACT exp latency
                    pts = []

                    def emit_lo(j):
                        p_pt, p_b, p_c0 = pts[j]
                        ap = p_pt[:, p_b + p_c0:p_b + 512]
                        nc.tensor.matmul(l_ps[:, p_c0:512], ones_sb[:], ap,
                                         start=(j == 0), stop=(j == njt - 1))
                        nc.tensor.matmul(oT_ps[:, p_c0:512],
                                         v_sb[:, bass.ts(j, 128)], ap,
                                         start=(j == 0), stop=(j == njt - 1))

                    st01 = None
                    defer = False
                    for jt in range(njt):
                        r = jt - 4 * g
                        c0 = max(r, 0) * 128   # first live column in i-chunk
                        isl = bass.ds(512 * g + c0, 512 - c0)
                        slot = jt % 3
                        if slot == 2:
                            st_t, st_b = pbank(2, name="st"), 0
                        else:
                            if slot == 0:
                                st01 = pbank01(name="st")
                            st_t, st_b = st01, slot * 512
                        nc.tensor.matmul(st_t[:, st_b + c0:st_b + 512],
                                         kTr[:, bass.ts(jt, 128)],
                                         qTr[:, h, isl], start=True, stop=True)
                        if slot == 0 and jt + 1 <= 4 * g - 1:
                            # next tile lands in the adjacent half of the same
                            # fused bank pair; one exp will cover both
                            defer = True
                            pts.append(None)   # placeholder, filled at jt+1
                        elif defer:
                            # paired exp over both halves (both full tiles)
                            pt2 = ptp.tile([128, 1024], bf16, tag="pt2", bufs=3)
                            nc.scalar.activation(pt2[:], st01[:],
                                                 mybir.ActivationFunctionType.Exp,
                                                 scale=SCALE)
                            pts[jt - 1] = (pt2, 0, 0)
                            pts.append((pt2, 512, 0))
                            defer = False
                        else:
                            pt = ptp.tile([128, 512], bf16, tag="pt", bufs=4)
                            nc.scalar.activation(pt[:, c0:512],
                                                 st_t[:, st_b + c0:st_b + 512],
                                                 mybir.ActivationFunctionType.Exp,
                                                 scale=SCALE)
                            if r >= 0:
                                # causal mask, first live 128 cols (0/1 mul)
                                nc.vector.tensor_mul(pt[:, c0:c0 + 128],
                                                     pt[:, c0:c0 + 128],
                                                     tri_sb[:])
                            pts.append((pt, 0, c0))
                        if jt >= LAG:
                            emit_lo(jt - LAG)
                        if jt == min(2, njt - 1):
                            # previous head's epilogue, now that the PE has
                            # runway (its 1/l ACT chain is long done)
                            flush_epilogue()
                    for j in range(max(njt - LAG, 0), njt):
                        emit_lo(j)
                    # 1/l = exp(-ln l) entirely on ACT: keeps the in-order ACT
                    # queue free of cross-engine waits; the PE-side broadcast
                    # and normalize are deferred one head
                    lnl = epi.tile([1, 512], f32, tag="lnl")
                    nc.scalar.activation(lnl[:], l_ps[:],
                                         mybir.ActivationFunctionType.Ln)
                    linv_bf = epi.tile([1, 512], bf16, tag="linvbf")
                    nc.scalar.activation(linv_bf[:], lnl[:],
                                         mybir.ActivationFunctionType.Exp,
                                         scale=-1.0)
                    pending.append((g, h, oT_ps, linv_bf))
                if n == NSEQ - 1:
                    # no next chunk to carry the deferred epilogue
                    flush_epilogue()

            # ---- phase C: outT = wo_c^T @ Y^T, wo stationary from SBUF ----
            for ns in range(NSEQ):
                if ns % 2 == 0:
                    o01 = pbank01(name="o")
                    o_slots = [(o01, 0), (o01, 512), (pbank(2, name="o"), 0),
                               (pbank(3, name="o"), 0)]
                else:
                    o_slots = [(pbank(4 + ob, name="o"), 0) for ob in range(QH)]
                for kt in range(KT):
                    c, db = kt // 4, kt % 4
                    y_sb = cproj.tile([128, 512], bf16, tag="y", bufs=10)
                    nc.sync.dma_start(y_sb[:], ag_outs[ns][c, bass.ts(db, 128)])
                    for ob in range(QH):
                        ot, obase = o_slots[ob]
                        nc.tensor.matmul(
                            ot[:, obase:obase + 512],
                            wo_sb[:, kt, bass.ts(ob, 128)], y_sb[:],
                            start=(kt == 0), stop=(kt == KT - 1))
                for ob in range(QH):
                    ot, obase = o_slots[ob]
                    o_sb = cproj.tile([128, 512], bf16, tag="osb", bufs=4)
                    nc.scalar.copy(o_sb[:], ot[:, obase:obase + 512])
                    nc.sync.dma_start(outT[bass.ts(ob, 128), bass.ts(ns, 512)],
                                      o_sb[:])

    _legalize_waits(nc)
    return nc


def _host_inputs(x, wq, wk, wv, wo):
    x = np.asarray(x, dtype=np.float32)
    xT = np.ascontiguousarray(x.reshape(S, DIM).T).astype(ml_dtypes.bfloat16)

    # rope tables in [hd, seq] layout with the sign of sin baked in
    inv_freq = 1.0 / ROPE_BASE ** (np.arange(0, HD, 2, dtype=np.float32) / HD)
    t = np.arange(S, dtype=np.float32)
    freqs = np.outer(inv_freq, t)                       # [64, S]
    cosT = np.concatenate([np.cos(freqs), np.cos(freqs)], 0)
    sinT = np.concatenate([-np.sin(freqs), np.sin(freqs)], 0)

    # 0/1 causal mask for a 128x128 diagonal block: keep j <= i
    j = np.arange(128)[:, None]
    i = np.arange(128)[None, :]
    tri = (j <= i).astype(np.float32)

    perm = np.zeros((128, 128), dtype=np.float32)
    perm[np.arange(128), (np.arange(128) + 64) % 128] = 1.0
    ident = np.eye(128, dtype=np.float32)

    shared = {
        "xT": xT,
        "cosT": cosT.astype(ml_dtypes.bfloat16),
        "sinT": sinT.astype(ml_dtypes.bfloat16),
        "tri": tri.astype(ml_dtypes.bfloat16),
        "perm": perm.astype(ml_dtypes.bfloat16),
        "ident": ident.astype(ml_dtypes.bfloat16),
    }
    maps = []
    for c in range(NCORES):
        m = dict(shared)
        m["wq"] = np.asarray(wq[:, c * QW:(c + 1) * QW]).astype(ml_dtypes.bfloat16)
        m["wk"] = np.asarray(wk[:, c * HD:(c + 1) * HD]).astype(ml_dtypes.bfloat16)
        m["wv"] = np.asarray(wv[:, c * HD:(c + 1) * HD]).astype(ml_dtypes.bfloat16)
        m["wo"] = np.asarray(wo[:, c * QW:(c + 1) * QW]).astype(ml_dtypes.bfloat16)
        maps.append(m)
    return maps


LAST_RESULT = {}


def kernel(x, wq, wk, wv, wo, mask=None, trace=False):
    if "nc" not in _CACHE:
        _CACHE["nc"] = _build_nc()
    nc = _CACHE["nc"]
    in_maps = _host_inputs(x, wq, wk, wv, wo)
    res = run_bass_kernel_spmd(nc, in_maps, list(range(NCORES)), trace=trace)
    LAST_RESULT["exec_time_ns"] = res.exec_time_ns
    LAST_RESULT["profile_json"] = res.profile_json
    it = res.instructions_and_trace
    LAST_RESULT["trace_dir"] = it if isinstance(it, str) else None
    full = np.concatenate(
        [res.results[c]["outT"].astype(np.float32).T for c in range(NCORES)],
        axis=1)
    return np.ascontiguousarray(full).reshape(1, S, DIM).astype(np.float32)
